# revision 41
# baseline (speedup 1.0000x reference)
"""Causal self-attention (B=4, L=2048, D=1536, H=24, RoPE) on 8 trn2 NeuronCores.

Sharding: hybrid batch x head-group tensor parallel. Core c handles batch
c//2 with head group c%2 (12 of 24 heads). Partial output projections are
summed pairwise with a 2-rank ReduceScatter over the feature dim; the host
transposes/concats.

v3: fully unrolled (no tc.For_i back-edges, ~2us each), all-bf16 matmuls
(fp32->bf16 casts ride the DMA via SWDGE), causal skip (only key chunks
j <= diagonal are computed: 40/64 of blocks), mask multiply only on the
4 diagonal key-chunks per query tile, softmax reciprocal via
reciprocal_approx_fast (5x faster than the iterative DVE reciprocal), all
PSUM->SBUF copies on DVE so the Scalar engine stays loaded with the Exp
table, and the tail ReduceScatter split in two token-halves so the first
half overlaps the output projection.

Pipeline per core:
  Q  qk projection: for each of 12 feature chunks m (6 q head-pairs then
     6 k head-pairs), 12 K-chunk matmuls per 512-token tile accumulate
     w_qk^T x in PSUM; RoPE = rotation matmul + cos/sin muls, bf16 out.
  X  v projection into the PV lhsT layout vb[128 tok, hp, tc, 130] with
     ones columns at 64/129 so the softmax denominator falls out of the
     PV matmul (row 64).
  B  attention per (head pair, 512-query tile): scores k^T q for key
     chunks 0..4iv+3 (two heads row-packed at array rows 0-63/64-127),
     exp (scale folded) to bf16, diagonal-chunk masking by bf16 multiply,
     PV accumulation (M=65), normalization via approx reciprocal of
     row 64 + gpsimd partition broadcast.
  C  output projection per (512-token tile, 128-feature chunk):
     partial^T[of, tok] += wout^T attn, bf16 operands, bf16 partial.
Tail: pairwise ReduceScatter(add) over partial^T in two token halves,
     rank g keeps rows g*768..(g+1)*768 (output features), host assembles.
"""

import sys

sys.path.insert(0, "/opt/trn_rl_repo")

import numpy as np

import concourse.bass as bass
import concourse.mybir as mybir
import concourse.tile as tile
from concourse import bacc
from concourse.bass_utils import run_bass_kernel_spmd

P = 128
B, L, D = 4, 2048, 1536
H, DH = 24, 64
HL = 12            # heads per core
NPAIR = 6          # head pairs per core
KC = D // P        # 12 contraction chunks for D
QF = HL * DH       # 768 q (or k) features per core
NT = L // 512      # 4 token tiles of 512
TC = L // P        # 16 token chunks of 128
LH = L // 2        # token half for the split ReduceScatter
ROPE_BASE = 10000.0

f32 = mybir.dt.float32
bf16 = mybir.dt.bfloat16

_CACHE = {}
LAST_RESULT = None


def _build_nc(debug_dumps=False):
    nc = bacc.Bacc(
        "TRN2",
        target_bir_lowering=False,
        debug=False,
        num_devices=8,
    )

    xT = nc.dram_tensor("xT", [D, L], bf16, kind="ExternalInput")
    wqk4 = nc.dram_tensor("wqk4", [P, KC, 2 * NPAIR, P], bf16, kind="ExternalInput")
    wvT = nc.dram_tensor("wvT", [D, QF], bf16, kind="ExternalInput")
    wout2 = nc.dram_tensor("wout2", [P, NPAIR, D], bf16, kind="ExternalInput")
    cosT = nc.dram_tensor("cosT", [P, L], bf16, kind="ExternalInput")
    sinT = nc.dram_tensor("sinT", [P, L], bf16, kind="ExternalInput")
    rotT = nc.dram_tensor("rotT", [P, P], bf16, kind="ExternalInput")
    maskT = nc.dram_tensor("maskT", [P, 2, 2, 512], bf16, kind="ExternalInput")
    out_ext = nc.dram_tensor("out", [4, D, 256], bf16, kind="ExternalOutput")
    if debug_dumps:
        qk_dump = nc.dram_tensor("qk_dump", [P, 2 * NPAIR, L], bf16,
                                 kind="ExternalOutput")
        vb_dump = nc.dram_tensor("vb_dump", [P, NPAIR, TC, 130], bf16,
                                 kind="ExternalOutput")
        attn_dump = nc.dram_tensor("attn_dump", [P, NPAIR, L], bf16,
                                   kind="ExternalOutput")
        den_dump = nc.dram_tensor("den_dump", [1, NPAIR, NT, 2, 512], f32,
                                  kind="ExternalOutput")
        rc_dump = nc.dram_tensor("rc_dump", [1, NPAIR, NT, 2, 512], f32,
                                 kind="ExternalOutput")

    Exp = mybir.ActivationFunctionType.Exp
    xT_r = xT.rearrange("(kc p) t -> p kc t", p=P)
    wvT_r = wvT.rearrange("(kc p) f -> p kc f", p=P)

    with tile.TileContext(nc) as tc:
        with tc.tile_pool(name="dram", bufs=1, space="DRAM") as dram:
            # [eighth, D, 256]: RS over a contiguous even-count block of
            # eighths scatters by token range (rank 2b keeps the first
            # half); RS_a covers tokens 0..1535, RS_b the final 512
            partial8 = dram.tile([8, D, 256], bf16)
            rs8 = dram.tile([4, D, 256], bf16)
            partial8_r = partial8.rearrange("h (c p) t -> p h c t", p=P)

            from contextlib import ExitStack

            with ExitStack() as stack:
                permq = stack.enter_context(tc.tile_pool(name="permq", bufs=1))
                qk_sb = permq.tile([P, 2 * NPAIR, L], bf16, tag="qk")
                permv = stack.enter_context(tc.tile_pool(name="permv", bufs=1))
                vb = permv.tile([P, NPAIR, TC, 130], bf16, tag="vb")
                perma = stack.enter_context(tc.tile_pool(name="perma", bufs=1))
                attn = perma.tile([P, NPAIR, L], bf16, tag="attn")

                # ---------------- Q + X: projections ----------------
                with (
                    tc.tile_pool(name="qx", bufs=1) as qxp,
                    tc.tile_pool(name="qc", bufs=1) as qcp,
                ):
                    x_bf = qxp.tile([P, KC, L], bf16, tag="x")

                    # ---- Q: qk projection + RoPE ----
                    with (
                        tc.tile_pool(name="qw", bufs=3) as qwp,
                        tc.tile_pool(name="qs", bufs=4) as qsp,
                        tc.tile_pool(name="qt", bufs=6) as qtp,
                        tc.tile_pool(name="qps", bufs=3, space="PSUM") as qps,
                        tc.tile_pool(name="qpr", bufs=3, space="PSUM") as qpr,
                    ):
                        # first weight chunk ahead of x in the HWDGE FIFO so
                        # the first matmul gates only on x slice 0
                        w_first = qwp.tile([P, KC, P], bf16, name="w_t")
                        nc.sync.dma_start(w_first[:], wqk4[:, :, 0, :])
                        cos_sb = qcp.tile([P, L], bf16, tag="cos")
                        sin_sb = qcp.tile([P, L], bf16, tag="sin")
                        rot_sb = qcp.tile([P, P], bf16, tag="rot")
                        for t in range(NT):  # token-split x load
                            tsl = slice(t * 512, (t + 1) * 512)
                            nc.sync.dma_start(x_bf[:, :, tsl], xT_r[:, :, tsl])
                            if t == 1:
                                nc.sync.dma_start(cos_sb[:], cosT[:])
                                nc.sync.dma_start(sin_sb[:], sinT[:])
                                nc.sync.dma_start(rot_sb[:], rotT[:])
                        for m in range(2 * NPAIR):
                            if m == 0:
                                w_t = w_first
                            else:
                                w_t = qwp.tile([P, KC, P], bf16)
                                nc.sync.dma_start(w_t[:], wqk4[:, :, m, :])
                            for n in range(NT):
                                tsl = slice(n * 512, (n + 1) * 512)
                                ps = qps.tile([P, 512], f32)
                                for k in range(KC):
                                    nc.tensor.matmul(
                                        ps[:],
                                        w_t[:, k, :],
                                        x_bf[:, k, tsl],
                                        start=(k == 0),
                                        stop=(k == KC - 1),
                                    )
                                stage = qsp.tile([P, 512], bf16)
                                nc.scalar.copy(stage[:], ps[:])
                                prot = qpr.tile([P, 512], f32)
                                nc.tensor.matmul(
                                    prot[:], rot_sb[:], stage[:],
                                    start=True, stop=True,
                                )
                                t1 = qtp.tile([P, 512], bf16)
                                nc.vector.tensor_mul(t1[:], stage[:], cos_sb[:, tsl])
                                t2 = qtp.tile([P, 512], bf16)
                                nc.vector.tensor_mul(t2[:], prot[:], sin_sb[:, tsl])
                                nc.vector.tensor_add(qk_sb[:, m, tsl], t1[:], t2[:])

                    # ---- X: v projection into PV lhsT layout ----
                    with (
                        tc.tile_pool(name="wv", bufs=1) as wvp,
                        tc.tile_pool(name="vps", bufs=3, space="PSUM") as vps,
                    ):
                        wv_bf = wvp.tile([P, KC, QF], bf16, tag="wvb")
                        nc.sync.dma_start(wv_bf[:], wvT_r)
                        nc.vector.memset(vb[:, :, :, 64:65], 1.0)
                        nc.vector.memset(vb[:, :, :, 129:130], 1.0)
                        for t in range(TC):
                            tsl = slice(t * P, (t + 1) * P)
                            psv = [vps.tile([P, 384], f32, name=f"vpsn{h}")
                                   for h in range(2)]
                            for k in range(KC):
                                for h in range(2):
                                    nc.tensor.matmul(
                                        psv[h][:],
                                        x_bf[:, k, tsl],
                                        wv_bf[:, k, 384 * h : 384 * (h + 1)],
                                        start=(k == 0),
                                        stop=(k == KC - 1),
                                    )
                            for h in range(2):
                                pr = psv[h].rearrange("p (a b) -> p a b", a=3)
                                nc.scalar.copy(
                                    vb[:, 3 * h : 3 * h + 3, t : t + 1, 0:64],
                                    pr[:, :, None, 0:64],
                                )
                                nc.scalar.copy(
                                    vb[:, 3 * h : 3 * h + 3, t : t + 1, 65:129],
                                    pr[:, :, None, 64:128],
                                )

                # ---------------- B + C: attention fused with output
                # projection.  iv-outer so each 512-token attention tile
                # completes across all head pairs, then its output-projection
                # chunks interleave into the NEXT tile's PE stream (the
                # Scalar engine keeps exp-ing while the PE runs projection).
                # ReduceScatter fires per token-half as soon as its partial
                # quarters are stored.
                with (
                    tc.tile_pool(name="bm", bufs=1) as bmp,
                    tc.tile_pool(name="cw", bufs=1) as cwp,
                    tc.tile_pool(name="bpt", bufs=4) as bptp,
                    tc.tile_pool(name="brc", bufs=2) as brcp,
                    tc.tile_pool(name="bbc", bufs=2) as bbcp,
                    tc.tile_pool(name="co", bufs=3) as cop,
                    tc.tile_pool(name="bps", bufs=2, space="PSUM") as bps,
                    tc.tile_pool(name="bpo", bufs=2, space="PSUM") as bpo,
                ):
                    mask_sb = bmp.tile([P, 2, 2, 512], bf16, tag="mask")
                    nc.sync.dma_start(mask_sb[:], maskT[:])
                    wout_bf = cwp.tile([P, NPAIR, D], bf16, tag="woutb")
                    nc.sync.dma_start(wout_bf[:], wout2[:])

                    def emit_c_chunk(n, ofc):
                        tsl = slice(n * 512, (n + 1) * 512)
                        ps3f = bps.tile([P, 2, 512], f32, name="pssn")
                        ps3 = ps3f[:, 0, :]
                        for k in range(NPAIR):
                            nc.tensor.matmul(
                                ps3,
                                wout_bf[:, k, ofc * P : (ofc + 1) * P],
                                attn[:, k, tsl],
                                start=(k == 0),
                                stop=(k == NPAIR - 1),
                            )
                        po = cop.tile([P, 512], bf16)
                        nc.vector.tensor_copy(po[:], ps3)
                        nc.sync.dma_start(
                            partial8_r[:, 2 * n : 2 * n + 2, ofc, :],
                            po.rearrange("p (e t) -> p e t", e=2),
                        )

                    RS_PARTS = [(0, 6, 0, 3), (6, 8, 3, 4)]

                    def emit_rs(h):
                        lo, hi, olo, ohi = RS_PARTS[h]
                        nc.gpsimd.collective_compute(
                            "ReduceScatter",
                            mybir.AluOpType.add,
                            replica_groups=[[0, 1], [2, 3], [4, 5], [6, 7]],
                            ins=[partial8[lo:hi].opt()],
                            outs=[rs8[olo:ohi].opt()],
                        )
                        nc.sync.dma_start(out_ext[olo:ohi], rs8[olo:ohi])

                    c_pending = []  # (quarter, ofc) chunks ready to emit
                    for iv in range(NT):
                        qsl = slice(iv * 512, (iv + 1) * 512)
                        jlast = 4 * iv + 3
                        # spread pending C chunks across this block's units
                        n_units = (2 * iv + 2) * 2 * NPAIR
                        c_every = max(2, (6 * n_units // 10)
                                      // max(1, len(c_pending))) \
                            if c_pending else 0
                        ucount = 0
                        for hp in range(NPAIR):
                            pso = [
                                bpo.tile([65, 512], f32, name=f"pson{hh}")
                                for hh in range(2)
                            ]
                            prev = None
                            for pr_i in range(2 * iv + 2):
                                diag = pr_i - 2 * iv  # >= 0 on the diagonal
                                for hh in range(2):
                                    off = 64 * hh
                                    pss = bps.tile([P, 2, 512], f32, name="pssn")
                                    for jj in range(2):
                                        jc = 2 * pr_i + jj
                                        nc.tensor.matmul(
                                            pss[:, jj, :],
                                            qk_sb[off : off + 64, NPAIR + hp,
                                                  jc * P : (jc + 1) * P],
                                            qk_sb[off : off + 64, hp, qsl],
                                            start=True,
                                            stop=True,
                                        )
                                    pt = bptp.tile([P, 2, 512], bf16)
                                    nc.scalar.activation(
                                        pt.rearrange("p a b -> p (a b)"),
                                        pss.rearrange("p a b -> p (a b)"),
                                        Exp,
                                        scale=0.125,
                                    )
                                    if diag >= 0:
                                        nc.vector.tensor_mul(
                                            pt[:], pt[:], mask_sb[:, diag, :, :]
                                        )
                                    if prev is not None:
                                        ppr, phh, ppt = prev
                                        for jj in range(2):
                                            jc = 2 * ppr + jj
                                            nc.tensor.matmul(
                                                pso[phh][:],
                                                vb[:, hp, jc,
                                                   65 * phh : 65 * phh + 65],
                                                ppt[:, jj, :],
                                                start=(jc == 0),
                                                stop=(jc == jlast),
                                            )
                                    prev = (pr_i, hh, pt)
                                    ucount += 1
                                    if c_pending and c_every \
                                            and ucount % c_every == 0:
                                        emit_c_chunk(*c_pending.pop(0))
                                        if not c_pending and iv == 3:
                                            emit_rs(0)
                            ppr, phh, ppt = prev
                            for jj in range(2):
                                jc = 2 * ppr + jj
                                nc.tensor.matmul(
                                    pso[phh][:],
                                    vb[:, hp, jc, 65 * phh : 65 * phh + 65],
                                    ppt[:, jj, :],
                                    start=(jc == 0),
                                    stop=(jc == jlast),
                                )
                            for hh in range(2):
                                off = 64 * hh
                                den_sb = brcp.tile([1, 512], f32, tag="den")
                                nc.vector.tensor_copy(
                                    den_sb[:], pso[hh][64:65, :]
                                )
                                rc = brcp.tile([1, 512], f32, tag="rc")
                                nc.vector.reciprocal_approx_fast(
                                    rc[:], den_sb[:]
                                )
                                rbc = bbcp.tile([64, 512], f32, tag="rbc")
                                nc.gpsimd.partition_broadcast(rbc[:], rc[:])
                                nc.vector.tensor_mul(
                                    attn[off : off + 64, hp, qsl],
                                    pso[hh][0:64, :],
                                    rbc[:],
                                )
                        # leftover chunks not drained during the block
                        while c_pending:
                            emit_c_chunk(*c_pending.pop(0))
                        c_pending = [(iv, ofc) for ofc in range(KC)]
                    for n_ofc in c_pending:
                        emit_c_chunk(*n_ofc)
                    emit_rs(1)

    nc.compile()
    return nc


def _rope_tables(pos_offset):
    import ml_dtypes
    inv_freq = 1.0 / (ROPE_BASE ** (np.arange(0, DH, 2, dtype=np.float32) / DH))
    t = np.arange(L, dtype=np.float32) + np.float32(pos_offset)
    freqs = np.outer(t, inv_freq)                      # (L, 32)
    emb = np.concatenate([freqs, freqs], axis=-1)      # (L, 64)
    cosT = np.cos(emb).T.astype(np.float32)            # (64, L)
    sinT = np.sin(emb).T.astype(np.float32)
    cos2 = np.concatenate([cosT, cosT], axis=0)        # (128, L)
    sin2 = np.concatenate([sinT, sinT], axis=0)
    return (
        np.ascontiguousarray(cos2).astype(ml_dtypes.bfloat16),
        np.ascontiguousarray(sin2).astype(ml_dtypes.bfloat16),
    )


def _rot_matrix():
    import ml_dtypes
    R = np.zeros((DH, DH), dtype=np.float32)
    R[:32, 32:] = -np.eye(32, dtype=np.float32)
    R[32:, :32] = np.eye(32, dtype=np.float32)
    R2 = np.zeros((P, P), dtype=np.float32)
    R2[:64, :64] = R
    R2[64:, 64:] = R
    return np.ascontiguousarray(R2.T).astype(ml_dtypes.bfloat16)


def _masks():
    import ml_dtypes
    # mask[k, t, q] = (t*128 + k <= q) for the 4 diagonal key chunks
    kr = np.arange(P)[:, None, None]
    tr = np.arange(4)[None, :, None] * P
    qr = np.arange(512)[None, None, :]
    m = (tr + kr <= qr).astype(np.float32)             # (128, 4, 512)
    return np.ascontiguousarray(m.reshape(P, 2, 2, 512)).astype(ml_dtypes.bfloat16)


def _make_in_maps(x, w_qkv, w_out, pos_offset):
    import ml_dtypes
    bf = ml_dtypes.bfloat16
    x = np.asarray(x, dtype=np.float32)
    w_qkv = np.asarray(w_qkv, dtype=np.float32)
    w_out = np.asarray(w_out, dtype=np.float32)

    cos2, sin2 = _rope_tables(int(pos_offset))
    rotT = _rot_matrix()
    maskT = _masks()

    in_maps = []
    for c in range(8):
        b, g = c // 2, c % 2
        rows_q = slice(g * QF, (g + 1) * QF)
        rows_k = slice(D + g * QF, D + (g + 1) * QF)
        rows_v = slice(2 * D + g * QF, 2 * D + (g + 1) * QF)
        wqkT = np.concatenate([w_qkv[rows_q], w_qkv[rows_k]], axis=0).T  # (D, 2QF)
        # (128, kc, m, 128): wqk4[p, kc, m, f] = wqkT[kc*128+p, m*128+f]
        wqk4 = np.ascontiguousarray(
            wqkT.reshape(KC, P, 2 * NPAIR, P).transpose(1, 0, 2, 3)
        ).astype(bf)
        wvT = np.ascontiguousarray(w_qkv[rows_v].T).astype(bf)   # (D, QF)
        # (128, hp, of): wout2[p, hp, of] = w_out[of, g*QF + hp*128 + p]
        wout2 = np.ascontiguousarray(
            w_out[:, g * QF : (g + 1) * QF].T.reshape(NPAIR, P, D).transpose(1, 0, 2)
        ).astype(bf)
        xT = np.ascontiguousarray(x[b].T).astype(bf)             # (D, L)
        in_maps.append(
            {
                "xT": xT,
                "wqk4": wqk4,
                "wvT": wvT,
                "wout2": wout2,
                "cosT": cos2,
                "sinT": sin2,
                "rotT": rotT,
                "maskT": maskT,
            }
        )
    return in_maps


def _assemble(results):
    # RS_a scatters tokens 0..1535: rank 2b keeps 0..767, rank 2b+1
    # keeps 768..1535.  RS_b scatters tokens 1536..2047: rank 2b keeps
    # 1536..1791, rank 2b+1 keeps 1792..2047.
    # part0 = tokens 0..1535 (rank even keeps 0..767), part1 = quarter 3
    # (rank even keeps tokens 1536..1791)
    out = np.empty((B, L, D), dtype=np.float32)
    for b in range(B):
        for g in range(2):
            r = results[2 * b + g]["out"].astype(np.float32)  # (4, D, 256)
            t0 = g * 768
            out[b, t0 : t0 + 768, :] = (
                r[0:3].transpose(0, 2, 1).reshape(768, D)
            )
            t1 = 1536 + g * 256
            out[b, t1 : t1 + 256, :] = r[3].T
    return out


def kernel(x, w_qkv, w_out, pos_offset):
    global LAST_RESULT
    if "nc" not in _CACHE:
        _CACHE["nc"] = _build_nc()
    nc = _CACHE["nc"]
    in_maps = _make_in_maps(x, w_qkv, w_out, pos_offset)
    res = run_bass_kernel_spmd(nc, in_maps, list(range(8)))
    LAST_RESULT = res
    return _assemble(res.results)
